# revision 7
# baseline (speedup 1.0000x reference)
"""Trainium2 Bass kernel for a dense transformer block (LN->causal attn->res->LN->MLP->res).

Shapes (hardcoded): x [2, 2048, 1024], 16 heads, head_dim 64, MLP hidden 4096, fp32 out.

v5 sharding: 8 cores = (batch b in {0,1}) x (token class j in {0..3}).
Class j = tokens {t : t % 4 == j} (512 tokens, position order).  Each core:
  * computes LN1 + Q/K/V for ONLY its own 512 tokens,
  * AllGathers K^T and V_aug across the 4 cores of its batch
    (replica groups [[0..3],[4..7]]; gathered layout is class-major, identical
    on every core),
  * runs causal-skip attention: key tile (c, t) (class c, 128-token tile t)
    is needed only by query tiles q >= t, so tiles with t > q are never
    computed (saves 37.5% of score/exp/AV work, uniformly on every core --
    interleaving balances the causal triangle).  The ragged diagonal (q == t)
    is fixed by a per-core 0/1 mask (tri if key class <= own class else
    strictly-lower) multiplied after the exp.
  * computes LN2 + MLP + residuals for its 512 tokens and writes them out;
    the host scatters rows back to positions j::4.

Carried over from v3/v4: ONE packed uint8 input tensor (runtime staging costs
~63us per tensor + ~10us/MB per exec, dominating the wall clock); bf16
weights (fp8 fails the 2e-2 error gate); bf16-only x; V augmented with a
per-head ones column accumulating the softmax denominator.
"""

from contextlib import ExitStack

import numpy as np

import concourse.bacc as bacc
import concourse.mybir as mybir
import concourse.tile as tile
from concourse.masks import make_identity

F32 = mybir.dt.float32
BF16 = mybir.dt.bfloat16
FP8 = mybir.dt.float8e4
AF = mybir.ActivationFunctionType
ALU = mybir.AluOpType

B = 2
T = 2048
D = 1024
H = 16
HD = 64
HDA = HD + 1  # +1 denominator column per head
MLP = 4096
NQ = 512  # tokens per core
CTX = T
EPS = 1e-5

N_CORES = 8
P = 128
CLS = 4

D_T = D // P  # 8
Q_T = NQ // P  # 4 query tiles (also key tiles per class)
M_T = MLP // P  # 32
VA = H * HDA  # 1040 augmented V width

REPLICA_GROUPS = [[0, 1, 2, 3], [4, 5, 6, 7]]

# ---- packed-input layout (bytes). All segments 4KB-aligned. ----
def _align(x, a=4096):
    return (x + a - 1) // a * a


_off = 0
def _seg(nbytes):
    global _off
    o = _off
    _off = _align(_off + nbytes)
    return o


OFF_X = _seg(NQ * D * 2)            # bf16 [512,1024] own-class tokens
OFF_WQ = _seg(D * D * 2)            # bf16 [1024,1024] pretiled
OFF_WK = _seg(D * D * 2)            # bf16 [1024,1024] pretiled
OFF_WVA = _seg(D * VA * 2)          # bf16 [1024,1040]
OFF_WFC = _seg(MLP * D * 2)         # bf16 [4096,1024] pretiled
OFF_WPJ = _seg(D * MLP * 2)         # bf16 [1024,4096] pretiled
OFF_TRI = _seg(P * CLS * 2 * P)     # fp8 0/1 [128, 4, 2, 128]
OFF_BQK = _seg(P * 2 * D_T * 4)     # f32 [128,16]
OFF_BFC = _seg(P * M_T * 4)         # f32 [128,32]
OFF_BPJ = _seg(P * D_T * 4)         # f32 [128,8]
OFF_BVA = _seg(VA * 2)              # bf16 [1,1040]
NB = _align(_off)


def build_program(loop_n: int = 1, bv_nonzero: bool = False):
    """Emit the SPMD Bass program. Returns finalized nc."""
    nc = bacc.Bacc("TRN2", target_bir_lowering=False, num_devices=N_CORES)

    pk = nc.dram_tensor("pk", [1, NB], mybir.dt.uint8, kind="ExternalInput")
    out = nc.dram_tensor("out", [NQ, D], F32, kind="ExternalOutput")
    cc_kin = nc.dram_tensor("cc_kin", [P, D_T, NQ], BF16, kind="Internal")
    cc_kout = nc.dram_tensor("cc_kout", [CLS, P, D_T, NQ], BF16, kind="Internal")
    cc_vin = nc.dram_tensor("cc_vin", [P, Q_T, VA], BF16, kind="Internal")
    cc_vout = nc.dram_tensor("cc_vout", [CLS, P, Q_T, VA], BF16, kind="Internal")

    def view(off, nbytes, dt):
        return pk[0, off : off + nbytes].bitcast(dt)

    with tile.TileContext(nc) as tc:
        with ExitStack() as ctx:
            if loop_n > 1:
                ctx.enter_context(tc.For_i(0, loop_n, 1))
            const = ctx.enter_context(tc.tile_pool(name="const", bufs=1))
            identity = const.tile([P, P], F32)
            make_identity(nc, identity)
            identity_bf = const.tile([P, P], BF16)
            make_identity(nc, identity_bf)
            ones1 = const.tile([1, P], BF16)
            nc.vector.memset(ones1, 1.0)
            eps_t = const.tile([P, 1], F32)
            nc.vector.memset(eps_t, EPS)
            bqk_sb = const.tile([P, 2 * D_T], F32)
            nc.sync.dma_start(
                bqk_sb, view(OFF_BQK, P * 2 * D_T * 4, F32).rearrange("(p c) -> p c", p=P)
            )
            bva_sb = const.tile([1, VA], BF16)
            nc.sync.dma_start(
                bva_sb, view(OFF_BVA, VA * 2, BF16).rearrange("(p c) -> p c", p=1)
            )
            tri8 = const.tile([P, CLS, 2, P], FP8)
            nc.sync.dma_start(
                tri8,
                view(OFF_TRI, P * CLS * 2 * P, FP8).rearrange(
                    "(p c s q) -> p c s q", p=P, c=CLS, s=2
                ),
            )
            tri_sb = const.tile([P, CLS, 2, P], BF16)
            nc.vector.tensor_copy(tri_sb, tri8)

            # Long-lived pools.
            qt_cm = tc.tile_pool(name="qt", bufs=1)
            qt_pool = qt_cm.__enter__()
            QT = [qt_pool.tile([P, NQ], BF16, name=f"QT{i}") for i in range(D_T)]
            kt_cm = tc.tile_pool(name="ktp", bufs=1)
            kt_pool = kt_cm.__enter__()
            KTT = kt_pool.tile([P, D_T, CTX], BF16, name="KTT")
            vsb_cm = tc.tile_pool(name="vsb", bufs=1)
            vsb_pool = vsb_cm.__enter__()
            VSBT = vsb_pool.tile([P, CLS * Q_T, VA], BF16, name="VSBT")

            # RIGHT pools (live into P4/P5)
            yt_pool = ctx.enter_context(tc.tile_pool(name="yt", bufs=1, side="right"))
            YT = [yt_pool.tile([P, NQ], F32, name=f"YT{i}") for i in range(D_T)]
            x2_pool = ctx.enter_context(tc.tile_pool(name="x2", bufs=1, side="right"))
            X2 = [x2_pool.tile([P, D], F32, name=f"X2{i}") for i in range(Q_T)]
            l2t_pool = ctx.enter_context(
                tc.tile_pool(name="l2t", bufs=1, side="right")
            )
            L2T = [l2t_pool.tile([P, NQ], BF16, name=f"L2T{i}") for i in range(D_T)]
            xo_pool = ctx.enter_context(tc.tile_pool(name="xo", bufs=1, side="right"))
            XO = xo_pool.tile([P, Q_T, D], BF16, name="XO")
            # xnT on top of the RIGHT stack; freed after Q/K/V, wfc prefetch
            # reuses the space during attention.
            xnt_cm = tc.tile_pool(name="xnt", bufs=1, side="right")
            xnt_pool = xnt_cm.__enter__()
            xnT = [xnt_pool.tile([P, NQ], BF16, name=f"xnT{i}") for i in range(D_T)]

            # -------- P1: load own-class x, LN1, PE-transpose -> xnT ---------
            with tc.tile_pool(name="p1xn", bufs=4) as p1xn, tc.tile_pool(
                name="p1stat", bufs=6
            ) as p1s, tc.tile_pool(name="p1ps", bufs=4, space="PSUM") as p1ps:
                nc.sync.dma_start(
                    XO,
                    view(OFF_X, NQ * D * 2, BF16).rearrange(
                        "(a p c) -> p a c", p=P, c=D
                    ),
                )
                xns = []
                for tt in range(Q_T):
                    stats = p1s.tile([P, 2, 6], F32, tag="stats")
                    for g in range(2):
                        nc.vector.bn_stats(
                            stats[:, g, :], XO[:, tt, g * 512 : (g + 1) * 512]
                        )
                    mv = p1s.tile([P, 2], F32, tag="mv")
                    nc.vector.bn_aggr(mv, stats)
                    sd = p1s.tile([P, 1], F32, tag="sd")
                    nc.scalar.activation(sd, mv[:, 1:2], AF.Sqrt, bias=eps_t)
                    rstd = p1s.tile([P, 1], F32, tag="rstd")
                    nc.vector.reciprocal(rstd, sd)
                    nmb = p1s.tile([P, 1], F32, tag="nmb")
                    nc.vector.tensor_scalar(
                        nmb, mv[:, 0:1], rstd, -1.0, ALU.mult, ALU.mult
                    )
                    xn = p1xn.tile([P, D], BF16, tag="xn")
                    nc.scalar.activation(
                        xn, XO[:, tt, :], AF.Identity, bias=nmb, scale=rstd
                    )
                    xns.append(xn)
                for dt_ in range(D_T):
                    tp = p1ps.tile([P, Q_T, P], BF16, tag="tp")
                    for ai in range(Q_T):
                        nc.tensor.transpose(
                            tp[:, ai, :],
                            xns[ai][:, dt_ * P : (dt_ + 1) * P],
                            identity_bf,
                        )
                    if dt_ % 2 == 0:
                        nc.vector.tensor_copy(xnT[dt_], tp)
                    else:
                        nc.scalar.copy(xnT[dt_], tp)

            # -------- P2k: own K^T -> DRAM -> AllGather ----------------------
            with tc.tile_pool(name="p2k", bufs=2) as p2k, tc.tile_pool(
                name="p2ko", bufs=1
            ) as p2ko, tc.tile_pool(name="p2kps", bufs=3, space="PSUM") as p2kps:
                KTo = p2ko.tile([P, D_T, NQ], BF16, name="KTo")
                for mt in range(D_T):
                    ws = p2k.tile([P, D_T, P], BF16, tag="wsk")
                    nc.sync.dma_start(
                        ws,
                        view(OFF_WK + mt * P * D * 2, P * D * 2, BF16).rearrange(
                            "(p a c) -> p a c", p=P, c=P
                        ),
                    )
                    ps = p2kps.tile([P, NQ], F32, tag="ps")
                    for kt_ in range(D_T):
                        nc.tensor.matmul(
                            ps,
                            ws[:, kt_, :],
                            xnT[kt_],
                            start=(kt_ == 0),
                            stop=(kt_ == D_T - 1),
                        )
                    nc.vector.tensor_scalar_add(
                        KTo[:, mt, :], ps, bqk_sb[:, D_T + mt : D_T + mt + 1]
                    )
                nc.sync.dma_start(cc_kin[:, :, :], KTo)
                nc.gpsimd.collective_compute(
                    "AllGather",
                    ALU.bypass,
                    replica_groups=REPLICA_GROUPS,
                    ins=[cc_kin[:, :, :]],
                    outs=[cc_kout[:, :, :, :]],
                )

            # -------- P2v: own V_aug -> DRAM -> AllGather --------------------
            vchunks = [(0, 512), (512, 512), (1024, VA - 1024)]
            with tc.tile_pool(name="p2vw", bufs=1) as p2vw, tc.tile_pool(
                name="p2vo", bufs=1
            ) as p2vo, tc.tile_pool(name="p2vps", bufs=2, space="PSUM") as p2vps:
                WVA = p2vw.tile([P, D_T, VA], BF16, name="wva")
                nc.sync.dma_start(
                    WVA,
                    view(OFF_WVA, D * VA * 2, BF16).rearrange(
                        "(a p c) -> p a c", p=P, c=VA
                    ),
                )
                Vo = p2vo.tile([P, Q_T, VA], BF16, name="Vo")
                for tt in range(Q_T):
                    for ci, (c0, cw) in enumerate(vchunks):
                        ps = p2vps.tile([P, 512], F32, tag="ps")
                        for kt_ in range(D_T):
                            nc.tensor.matmul(
                                ps[:, :cw],
                                xnT[kt_][:, tt * P : (tt + 1) * P],
                                WVA[:, kt_, c0 : c0 + cw],
                                start=(kt_ == 0),
                                stop=(kt_ == D_T - 1 and not bv_nonzero),
                            )
                        if bv_nonzero:
                            nc.tensor.matmul(
                                ps[:, :cw],
                                ones1,
                                bva_sb[:, c0 : c0 + cw],
                                start=False,
                                stop=True,
                            )
                        if ci % 2 == 0:
                            nc.vector.tensor_copy(
                                Vo[:, tt, c0 : c0 + cw], ps[:, :cw]
                            )
                        else:
                            nc.scalar.copy(Vo[:, tt, c0 : c0 + cw], ps[:, :cw])
                if not bv_nonzero:
                    ones_cols = Vo.rearrange("p t (h c) -> p t h c", c=HDA)[
                        :, :, :, HD : HD + 1
                    ]
                    nc.vector.memset(ones_cols, 1.0)
                nc.sync.dma_start(cc_vin[:, :, :], Vo)
                nc.gpsimd.collective_compute(
                    "AllGather",
                    ALU.bypass,
                    replica_groups=REPLICA_GROUPS,
                    ins=[cc_vin[:, :, :]],
                    outs=[cc_vout[:, :, :, :]],
                )

            # -------- P2q: Q^T (overlaps the collectives) --------------------
            with tc.tile_pool(name="p2q", bufs=2) as p2q, tc.tile_pool(
                name="p2qps", bufs=3, space="PSUM"
            ) as p2qps:
                for mt in range(D_T):
                    ws = p2q.tile([P, D_T, P], BF16, tag="wsq")
                    nc.sync.dma_start(
                        ws,
                        view(OFF_WQ + mt * P * D * 2, P * D * 2, BF16).rearrange(
                            "(p a c) -> p a c", p=P, c=P
                        ),
                    )
                    ps = p2qps.tile([P, NQ], F32, tag="ps")
                    for kt_ in range(D_T):
                        nc.tensor.matmul(
                            ps,
                            ws[:, kt_, :],
                            xnT[kt_],
                            start=(kt_ == 0),
                            stop=(kt_ == D_T - 1),
                        )
                    nc.scalar.activation(
                        QT[mt], ps, AF.Identity, bias=bqk_sb[:, mt : mt + 1]
                    )

            # -------- gather readback: K then V ------------------------------
            for r in range(CLS):
                nc.sync.dma_start(
                    KTT[:, :, r * NQ : (r + 1) * NQ],
                    cc_kout[r, :, :, :],
                )
            for r in range(CLS):
                nc.sync.dma_start(
                    VSBT[:, r * Q_T : (r + 1) * Q_T, :],
                    cc_vout[r, :, :, :],
                )

            # xnT consumed -> free; prefetch half of wfc during attention.
            xnt_cm.__exit__(None, None, None)
            MT_RES = M_T // 2
            wfc_cm = tc.tile_pool(name="wfcp_sb", bufs=1, side="right")
            wfc_pool = wfc_cm.__enter__()
            WFC = wfc_pool.tile([P, MT_RES, D_T, P], BF16, name="WFC")
            nc.sync.dma_start(
                WFC,
                view(OFF_WFC, MT_RES * P * D * 2, BF16).rearrange(
                    "(a p c) -> p a c", p=P, c=D
                ).rearrange("p a (k c) -> p a k c", c=P),
            )

            # -------- P3: causal-skip attention, key tile (c, t) -------------
            # key tile (class c, tile t) serves query tiles q in [t, 4); the
            # first 128 query columns (q == t) get the ragged tri mask.
            ptp_cm = tc.tile_pool(name="ptp", bufs=4)
            ptp = ptp_cm.__enter__()
            p3s_cm = tc.tile_pool(name="p3s", bufs=2)
            p3s = p3s_cm.__enter__()
            stps_cm = tc.tile_pool(name="stps", bufs=2, space="PSUM")
            stps = stps_cm.__enter__()
            yps_cm = tc.tile_pool(name="yps", bufs=2, space="PSUM")
            yps = yps_cm.__enter__()
            for hp in range(H // 2):
                yp = yps.tile([HDA, 2, NQ], F32, name=f"yp{hp}", tag="yp")
                for t in range(Q_T):
                    nqc = (Q_T - t) * P  # query columns t*128 .. 512
                    for c in range(CLS):
                        kti = c * Q_T + t
                        kcol = c * NQ + t * P
                        # fixed 512-wide halves keep each matmul's PSUM
                        # region inside one 2KB bank
                        st = stps.tile([P, 2, NQ], F32, tag="st")
                        for s in range(2):
                            nc.tensor.matmul(
                                st[:, s, :nqc],
                                KTT[s * HD : (s + 1) * HD, hp, kcol : kcol + P],
                                QT[hp][s * HD : (s + 1) * HD, t * P :],
                                start=True,
                                stop=True,
                                tile_position=(s * HD, 0),
                            )
                        pt = ptp.tile([P, 2, nqc], BF16, tag="pt")
                        nc.scalar.activation(pt, st[:, :, :nqc], AF.Exp)
                        nc.vector.tensor_mul(
                            pt[:, :, 0:P], pt[:, :, 0:P], tri_sb[:, c, :, :]
                        )
                        # start=True zeroes the whole 2KB PSUM bank (one
                        # bank per s), so later sub-range accumulations are
                        # against zeroed/accumulated state; one start at
                        # (t=0,c=0), one stop at (t=3,c=3) per bank.
                        for s in range(2):
                            h = 2 * hp + s
                            nc.tensor.matmul(
                                yp[:, s, t * P :],
                                VSBT[:, kti, h * HDA : (h + 1) * HDA],
                                pt[:, s, :],
                                start=(t == 0 and c == 0),
                                stop=(t == Q_T - 1 and c == CLS - 1),
                            )
                for s in range(2):
                    ysb = p3s.tile([HDA, NQ], F32, name=f"ysb{hp}_{s}", tag="ysb")
                    if s == 0:
                        nc.vector.tensor_copy(ysb, yp[:, s, :])
                    else:
                        nc.scalar.copy(ysb, yp[:, s, :])
                    recip = p3s.tile([1, NQ], F32, tag="recip")
                    nc.vector.reciprocal(recip, ysb[HD : HD + 1, :])
                    rb = p3s.tile([HD, NQ], F32, tag="rb")
                    nc.gpsimd.partition_broadcast(rb, recip)
                    nc.vector.tensor_mul(
                        YT[hp][s * HD : (s + 1) * HD, :], ysb[:HD, :], rb
                    )

            yps_cm.__exit__(None, None, None)
            stps_cm.__exit__(None, None, None)
            p3s_cm.__exit__(None, None, None)
            ptp_cm.__exit__(None, None, None)
            vsb_cm.__exit__(None, None, None)
            kt_cm.__exit__(None, None, None)
            qt_cm.__exit__(None, None, None)

            # ---------------- P4: residual + LN2 + transpose -----------------
            with tc.tile_pool(name="p4w", bufs=3) as p4w, tc.tile_pool(
                name="p4s", bufs=4
            ) as p4s, tc.tile_pool(name="p4ps", bufs=4, space="PSUM") as p4ps:
                for tt in range(Q_T):
                    for mt in range(D_T):
                        tp = p4ps.tile([P, P], F32, tag="tp")
                        nc.tensor.transpose(
                            tp, YT[mt][:, tt * P : (tt + 1) * P], identity
                        )
                        nc.vector.tensor_add(
                            X2[tt][:, mt * P : (mt + 1) * P],
                            XO[:, tt, mt * P : (mt + 1) * P],
                            tp,
                        )
                    stats = p4s.tile([P, 2, 6], F32, tag="stats2")
                    for g in range(2):
                        nc.vector.bn_stats(
                            stats[:, g, :], X2[tt][:, g * 512 : (g + 1) * 512]
                        )
                    mv = p4s.tile([P, 2], F32, tag="mv2")
                    nc.vector.bn_aggr(mv, stats)
                    sd = p4s.tile([P, 1], F32, tag="sd2")
                    nc.scalar.activation(sd, mv[:, 1:2], AF.Sqrt, bias=eps_t)
                    rstd = p4s.tile([P, 1], F32, tag="rstd2")
                    nc.vector.reciprocal(rstd, sd)
                    nmb = p4s.tile([P, 1], F32, tag="nmb2")
                    nc.vector.tensor_scalar(
                        nmb, mv[:, 0:1], rstd, -1.0, ALU.mult, ALU.mult
                    )
                    l2 = p4w.tile([P, D], BF16, tag="l2")
                    nc.scalar.activation(l2, X2[tt], AF.Identity, bias=nmb, scale=rstd)
                    for mt in range(D_T):
                        tp = p4ps.tile([P, P], BF16, tag="tpb")
                        nc.tensor.transpose(
                            tp, l2[:, mt * P : (mt + 1) * P], identity_bf
                        )
                        if mt % 2 == 0:
                            nc.vector.tensor_copy(
                                L2T[mt][:, tt * P : (tt + 1) * P], tp
                            )
                        else:
                            nc.scalar.copy(L2T[mt][:, tt * P : (tt + 1) * P], tp)

            # ---------------- P5: MLP + final residual ----------------
            with tc.tile_pool(name="h1t", bufs=1) as h1t_pool, tc.tile_pool(
                name="p5w", bufs=2
            ) as p5w, tc.tile_pool(name="p5o", bufs=1) as p5o, tc.tile_pool(
                name="p5ps", bufs=3, space="PSUM"
            ) as p5ps, tc.tile_pool(
                name="p5tps", bufs=4, space="PSUM"
            ) as p5tps:
                bfc_sb = p5o.tile([P, M_T], F32)
                nc.sync.dma_start(
                    bfc_sb,
                    view(OFF_BFC, P * M_T * 4, F32).rearrange("(p c) -> p c", p=P),
                )
                bproj_sb = p5o.tile([P, D_T], F32)
                nc.sync.dma_start(
                    bproj_sb,
                    view(OFF_BPJ, P * D_T * 4, F32).rearrange("(p c) -> p c", p=P),
                )
                OUT = p5o.tile([P, Q_T, D], F32, name="OUT")
                H1T = [h1t_pool.tile([P, NQ], BF16, name=f"H1T{i}") for i in range(M_T)]
                for mt in range(M_T):
                    if mt < MT_RES:
                        wfc_t = WFC[:, mt]
                    else:
                        wfc_t = p5w.tile([P, D_T, P], BF16, tag="wsf")
                        nc.sync.dma_start(
                            wfc_t,
                            view(OFF_WFC + mt * P * D * 2, P * D * 2, BF16).rearrange(
                                "(p k c) -> p k c", p=P, c=P
                            ),
                        )
                    ps = p5ps.tile([P, NQ], F32, tag="ps")
                    for kt_ in range(D_T):
                        nc.tensor.matmul(
                            ps,
                            wfc_t[:, kt_, :],
                            L2T[kt_],
                            start=(kt_ == 0),
                            stop=(kt_ == D_T - 1),
                        )
                    nc.scalar.activation(
                        H1T[mt], ps, AF.Relu, bias=bfc_sb[:, mt : mt + 1]
                    )
                wfc_cm.__exit__(None, None, None)
                for mt in range(D_T):
                    ws = p5w.tile([P, M_T, P], BF16, tag="wsp")
                    nc.sync.dma_start(
                        ws,
                        view(OFF_WPJ + mt * P * MLP * 2, P * MLP * 2, BF16).rearrange(
                            "(p a c) -> p a c", p=P, c=P
                        ),
                    )
                    ps = p5ps.tile([P, NQ], F32, tag="ps")
                    for kt_ in range(M_T):
                        nc.tensor.matmul(
                            ps,
                            ws[:, kt_, :],
                            H1T[kt_],
                            start=(kt_ == 0),
                            stop=(kt_ == M_T - 1),
                        )
                    mlpt = p5w.tile([P, NQ], F32, tag="mlpt")
                    nc.vector.tensor_scalar_add(mlpt, ps, bproj_sb[:, mt : mt + 1])
                    for tt in range(Q_T):
                        tp = p5tps.tile([P, P], F32, tag="tp")
                        nc.tensor.transpose(
                            tp, mlpt[:, tt * P : (tt + 1) * P], identity
                        )
                        nc.vector.tensor_add(
                            OUT[:, tt, mt * P : (mt + 1) * P],
                            X2[tt][:, mt * P : (mt + 1) * P],
                            tp,
                        )
                nc.sync.dma_start(
                    out.rearrange("(a p) c -> p a c", p=P), OUT
                )

    nc.finalize()
    return nc


_PROG = {}


def _get_program(bv_nonzero: bool = False):
    if bv_nonzero not in _PROG:
        _PROG[bv_nonzero] = build_program(bv_nonzero=bv_nonzero)
    return _PROG[bv_nonzero]


def _pretile(w, n_out_tiles, n_k_tiles):
    """[K, N] -> lhsT pre-tiled layout: row (mt*128+p), flat col (kt*128+c)
    holds w[kt*128 + p, mt*128 + c]."""
    K, N = w.shape
    assert K == n_k_tiles * P and N == n_out_tiles * P
    # axes (kt, p, mt, c) -> (mt, p, kt, c)
    return np.ascontiguousarray(
        w.reshape(n_k_tiles, P, n_out_tiles, P)
        .transpose(2, 1, 0, 3)
        .reshape(n_out_tiles * P, n_k_tiles * P)
    )


def make_in_maps(x, ln1_scale, ln1_shift, w_qkv, b_qkv, ln2_scale, ln2_shift,
                 w_fc, b_fc, w_proj, b_proj):
    """Host-side prep: fold LN affine into weights, prescale Q by 1/sqrt(hd),
    augment V with the ones column, pre-tile weights (bf16), build the
    per-core class-interleaved x slice + ragged-diagonal masks, and pack
    everything into one uint8 tensor per core."""
    import ml_dtypes

    bf16 = ml_dtypes.bfloat16
    fp8 = mybir.dt.np(FP8)

    x = np.asarray(x, np.float32)
    ln1_scale = np.asarray(ln1_scale, np.float32)
    ln1_shift = np.asarray(ln1_shift, np.float32)
    w_qkv = np.asarray(w_qkv, np.float32)
    b_qkv = np.asarray(b_qkv, np.float32)
    ln2_scale = np.asarray(ln2_scale, np.float32)
    ln2_shift = np.asarray(ln2_shift, np.float32)
    w_fc = np.asarray(w_fc, np.float32)
    b_fc = np.asarray(b_fc, np.float32)
    w_proj = np.asarray(w_proj, np.float32)
    b_proj = np.asarray(b_proj, np.float32)

    # fold LN1 affine into qkv weights
    w1 = ln1_scale[:, None] * w_qkv  # [D, 3D]
    b1 = b_qkv + ln1_shift @ w_qkv  # [3D]
    sc = 1.0 / np.sqrt(HD)
    wq = w1[:, :D] * sc
    bq = b1[:D] * sc
    wk = w1[:, D : 2 * D]
    bk = b1[D : 2 * D]
    wv = w1[:, 2 * D :]
    bv = b1[2 * D :]

    wqp_h = _pretile(wq, D_T, D_T).astype(bf16)
    wkp_h = _pretile(wk, D_T, D_T).astype(bf16)
    bqk_h = np.ascontiguousarray(
        np.concatenate([bq, bk]).reshape(2 * D_T, P).T
    )  # [128, 16] f32

    wva_h = np.zeros((D, VA), np.float32)
    bva_h = np.zeros((1, VA), np.float32)
    for h in range(H):
        wva_h[:, h * HDA : h * HDA + HD] = wv[:, h * HD : (h + 1) * HD]
        bva_h[0, h * HDA : h * HDA + HD] = bv[h * HD : (h + 1) * HD]
        bva_h[0, h * HDA + HD] = 1.0  # denominator ones column
    wva_h = wva_h.astype(bf16)
    bva_h = bva_h.astype(bf16)

    # fold LN2 affine into fc; pre-tile bf16
    wfc_f = ln2_scale[:, None] * w_fc
    wfcp_h = _pretile(wfc_f, M_T, D_T).astype(bf16)  # [4096, 1024]
    wprojp_h = _pretile(w_proj, D_T, M_T).astype(bf16)  # [1024, 4096]
    bfc_h = np.ascontiguousarray((b_fc + ln2_shift @ w_fc).reshape(M_T, P).T)
    bproj_h = np.ascontiguousarray(b_proj.reshape(D_T, P).T)  # [128, 8]

    def put(buf, off, arr):
        bts = np.ascontiguousarray(arr).view(np.uint8).reshape(-1)
        buf[off : off + bts.size] = bts

    base = np.zeros(NB, np.uint8)
    put(base, OFF_WQ, wqp_h)
    put(base, OFF_WK, wkp_h)
    put(base, OFF_WVA, wva_h)
    put(base, OFF_WFC, wfcp_h)
    put(base, OFF_WPJ, wprojp_h)
    put(base, OFF_BQK, bqk_h)
    put(base, OFF_BFC, bfc_h)
    put(base, OFF_BPJ, bproj_h)
    put(base, OFF_BVA, bva_h)

    # ragged diagonal masks: key index i (partition) vs query index col:
    # keep if i <= col (key class c <= own class j) else i < col.
    ii = np.arange(P)[:, None]
    qq = np.arange(P)[None, :]
    tri_inc = (ii <= qq).astype(np.float32)  # [128,128]
    tri_exc = (ii < qq).astype(np.float32)

    in_maps = []
    for core in range(N_CORES):
        b, j = divmod(core, CLS)
        xo = x[b, j::CLS]  # [512, 1024] own class tokens
        tri_h = np.empty((P, CLS, 2, P), np.float32)
        for c in range(CLS):
            m = tri_inc if c <= j else tri_exc
            tri_h[:, c, 0, :] = m
            tri_h[:, c, 1, :] = m
        pkc = base.copy()
        put(pkc, OFF_X, np.ascontiguousarray(xo.astype(bf16)))
        put(pkc, OFF_TRI, tri_h.astype(fp8))
        in_maps.append({"pk": pkc.reshape(1, NB)})
    return in_maps


def assemble_output(results):
    out = np.empty((B, T, D), np.float32)
    for core in range(N_CORES):
        b, j = divmod(core, CLS)
        out[b, j::CLS, :] = results[core]["out"]
    return out


def kernel(**inputs) -> np.ndarray:
    from concourse.bass_utils import run_bass_kernel_spmd

    in_maps = make_in_maps(**inputs)
    bva = np.frombuffer(
        in_maps[0]["pk"][0, OFF_BVA : OFF_BVA + VA * 2].tobytes(),
        dtype=mybir.dt.np(BF16),
    ).astype(np.float32)
    mask = np.ones(VA, bool)
    mask[HD::HDA] = False  # the ones columns
    nc = _get_program(bv_nonzero=bool(np.any(bva[mask] != 0.0)))
    res = run_bass_kernel_spmd(nc, in_maps, core_ids=list(range(N_CORES)))
    return assemble_output(res.results)
